# revision 1
# baseline (speedup 1.0000x reference)
"""Trainium2 Bass kernel for nn_GammaSpaceLayer.

SSM with fixed "gamma" transition A (bidiagonal), bilinear discretization,
kernel k[l] = C dA^l dB, FFT causal conv + D*u skip.  Computed as a chunked
linear scan (state dim H=64, chunk T=8):

  z_t[c]   = sum_{s<=t} dA^{t-s} dB u[c,s]          (intra, pair-packed)
  x_end[c] = A8 x_end[c-1] + z_7[c]                 (radix-4 log scan)
  y_t[c]   = C z_t[c] + C dA^{t+1} x_end[c-1] + D u_t[c]

All matmul operands are bf16 (1 cy/row on the PE at any N); accumulation in
fp32 PSUM.  Layout work (transposes to channel-major, t-major column order)
is done on the host; small input-dependent matrices (powers of dA etc.) are
precomputed on host in float64 and passed as inputs, so the Bass program is
input-independent.  Data-parallel over batch: 16 batches over 8 cores.
"""

import numpy as np
import ml_dtypes

import concourse.bass as bass
import concourse.mybir as mybir
import concourse.tile as tile
from concourse.vector_clock import ScopedClock
from concourse.bass_utils import run_bass_kernel_spmd

# problem constants (hardcoded per contract)
H, S = 64, 128          # state dim, io channel dim
B, L = 16, 2048         # full batch, seq len
N_CORES = 8
PB = B // N_CORES       # batches per core (2)
T = 8                   # chunk length
CK = L // T             # chunks per batch (256)
COLS = PB * CK          # state columns per core (512)
NTOK = PB * L           # token columns per core (4096)
DT_MIN, DT_MAX = 0.001, 0.1

F32 = mybir.dt.float32
BF = mybir.dt.bfloat16
BF_NP = ml_dtypes.bfloat16

# weight layout offsets
#  w128 (128 rows): [ GTX: 9 blocks of 64 (zero, G0..G7, transposed) | DD: 128 ]
GTX_OFF = 0
DD_OFF = 9 * H          # 576
W128_COLS = DD_OFF + S  # 704
#  w64 (64 rows): [ CAT: 8 blocks of 128 | CT: 128 ]
SCAN_LEVELS = [(1, (1, 2, 3)), (4, (1, 2, 3)), (16, (1, 2, 3)), (64, (1, 2, 3))]
N_SW = sum(len(ks) for _, ks in SCAN_LEVELS)  # 12
CAT_OFF = 0
CT_OFF = T * S          # 1024
W64_COLS = CT_OFF + S
#  w128b (128 rows): 12 blockdiag scan weight blocks of 128 cols
W128B_COLS = N_SW * S

N_WARM = 2              # PE warm-up matmuls (ramp p-state during input DMA)


class _TC(tile.TileContext):
    """TileContext whose tail drain splits multi-sem waits: this walrus
    build caps CTRL instructions at one sync-wait command."""

    def _drain_and_barrier(self, tick_clock, wait_clock):
        probe = self.nc.sync.drain()
        wait_clock.add_sem_waits(probe.ins, ScopedClock({None: tick_clock.global_clock}))
        si = probe.ins.sync_info
        if si is not None and si.on_wait and len(si.on_wait) > 1:
            waits = list(si.on_wait)
            probe.ins.sync_info = mybir.SyncInfo(
                on_wait=[waits[0]], on_update=list(si.on_update or []))
            for w in waits[1:]:
                d = self.nc.sync.drain()
                d.ins.sync_info = mybir.SyncInfo(on_wait=[w], on_update=[])
        self.nc.all_engine_barrier()
        assert self.sems is not None
        popped = self.nc._tile_sem_poison_stack.pop()
        assert popped is self._sem_poison
        self.nc.clear_and_free_semaphores(list(self.sems.allocated().values()))
        self.nc.all_engine_barrier()


def _split_multi_waits(nc):
    """This walrus build allows only ONE sync-wait command per instruction.
    Split extras onto same-engine InstEventSemaphore carriers inserted
    immediately before (engine program order preserves semantics)."""
    n = 0
    for f in nc.m.functions:
        for b in f.blocks:
            il = b.instructions
            i = 0
            while i < len(il):
                ins = il[i]
                si = ins.sync_info
                if si is not None and si.on_wait and len(si.on_wait) > 1:
                    waits = list(si.on_wait)
                    ins.sync_info = mybir.SyncInfo(
                        on_wait=[waits[-1]], on_update=list(si.on_update or []))
                    for j, w in enumerate(waits[:-1]):
                        ev = mybir.InstEventSemaphore(
                            name=f"{ins.name}_wsplit{j}", ins=[], outs=[])
                        ev.engine = ins.engine
                        ev.sync_info = mybir.SyncInfo(on_wait=[w], on_update=[])
                        il.insert(i, ev)
                        i += 1
                        n += 1
                i += 1
    return n


def _build():
    nc = bass.Bass()
    u_d = nc.dram_tensor("u", [S, NTOK], BF, kind="ExternalInput")      # (i, t b c)
    w128_d = nc.dram_tensor("W128", [S, W128_COLS], BF, kind="ExternalInput")
    w64_d = nc.dram_tensor("W64", [H, W64_COLS], BF, kind="ExternalInput")
    w128b_d = nc.dram_tensor("W128B", [S, W128B_COLS], BF, kind="ExternalInput")
    y_d = nc.dram_tensor("y", [S, NTOK], BF, kind="ExternalOutput")     # (o, t b c)

    with _TC(nc) as tc:
        with (
            tc.tile_pool(name="const", bufs=1) as cpool,
            tc.tile_pool(name="work", bufs=3, space="PSUM") as wpool,
            tc.tile_pool(name="ypsum", bufs=5, space="PSUM") as ypool,
        ):
            # ---- SBUF tiles ----
            u_sb = cpool.tile([S, NTOK], BF)
            w128 = cpool.tile([S, W128_COLS], BF)
            w64 = cpool.tile([H, W64_COLS], BF)
            w128b = cpool.tile([S, W128B_COLS], BF)
            xb1 = cpool.tile([H, CK], BF)             # batch-1 xprev window
            wsrc = cpool.tile([S, COLS], BF)          # warm-up source (zeros)
            # scan state, batches stacked on partitions: rows 0:64 = batch 0,
            # rows 64:128 = batch 1; cols [0:CK) zero pad, [CK:2CK) data.
            sc_a = cpool.tile([S, 2 * CK], BF)        # scan ping
            sc_b = cpool.tile([S, 2 * CK], BF)        # scan pong
            z_sb = cpool.tile([H, T * COLS], BF)      # intra states, per t
            y_sb = cpool.tile([S, NTOK], BF)          # output staging

            sa = sc_a[:]
            sb = sc_b[:]

            def useg(t):
                return u_sb[:, t * COLS:(t + 1) * COLS]

            def zseg(t):
                return z_sb[:, t * COLS:(t + 1) * COLS]

            def yseg(t):
                return y_sb[:, t * COLS:(t + 1) * COLS]

            # ---- memsets (gpsimd) ----
            nc.gpsimd.memset(wsrc[:], 0)
            nc.gpsimd.memset(sa[:, 0:CK], 0)
            nc.gpsimd.memset(sb[:, 0:CK], 0)

            # ---- DMAs (SP queue): weights for intra first, then u slices ----
            nc.sync.dma_start(u_sb[:, 0:2 * COLS], u_d[:, 0:2 * COLS])
            nc.sync.dma_start(w128[:], w128_d[:])
            nc.sync.dma_start(u_sb[:, 2 * COLS:4 * COLS], u_d[:, 2 * COLS:4 * COLS])
            nc.sync.dma_start(u_sb[:, 4 * COLS:6 * COLS], u_d[:, 4 * COLS:6 * COLS])
            nc.sync.dma_start(u_sb[:, 6 * COLS:7 * COLS], u_d[:, 6 * COLS:7 * COLS])
            nc.sync.dma_start(u_sb[:, 7 * COLS:8 * COLS], u_d[:, 7 * COLS:8 * COLS])
            nc.sync.dma_start(w128b[:], w128b_d[:])
            nc.sync.dma_start(w64[:], w64_d[:])

            # ---- PE warm-up: ramp p-state while DMAs land ----
            warm = wpool.tile([S, COLS], F32, tag="w")
            for i in range(N_WARM):
                nc.tensor.matmul(warm[:], wsrc[:, 0:S], wsrc[:],
                                 start=(i == 0), stop=(i == N_WARM - 1))
            nc.scalar.copy(y_sb[0:1, 0:1], warm[0:1, 0:1])  # keep tile "read"

            # ---- intra-chunk states, pair-packed: pair q holds (z_2q; z_2q+1)
            # lhsT for (q, s) = w128[:, 64*(2q-s+1) : 64*(2q-s+1)+128]
            # (contiguous [G_{2q-s} | G_{2q+1-s}] in the GTX layout).
            qt = {}

            def intra(q, s, start, stop):
                nc.tensor.matmul(
                    qt[q][:], w128[:, H * (2 * q - s + 1): H * (2 * q - s + 1) + S],
                    useg(s), start=start, stop=stop)

            # issue order: pair 3 (the scan seed) strictly first, chasing
            # the u slice DMAs; other pairs + y accumulators fill the scan.
            qt[3] = wpool.tile([S, COLS], F32, tag="w", name="q3")
            for s in range(T):
                intra(3, s, s == 0, s == T - 1)
            # scan seed (stacked): batch 0 columns -> partitions 0:64
            # (partition-offset copy, same pattern as the unstacked seed);
            # batch 1 columns are partition-aligned.
            nc.vector.tensor_copy(sa[0:H, CK:2 * CK], qt[3][H:S, 0:CK])
            nc.vector.tensor_copy(sa[H:S, CK:2 * CK], qt[3][H:S, CK:2 * CK])
            nc.vector.tensor_copy(zseg(7), qt[3][H:S, :])
            nc.scalar.copy(zseg(6), qt[3][0:H, :])

            yt = {}

            def dmm(t, pool):
                yt[t] = pool.tile([S, COLS], F32, tag="y" if pool is ypool else "w",
                                  name=f"yt{t}")
                nc.tensor.matmul(yt[t][:], w128[:, DD_OFF:DD_OFF + S], useg(t),
                                 start=True, stop=False)

            def cmm(t):
                nc.tensor.matmul(yt[t][:], w64[:, CT_OFF:CT_OFF + S], zseg(t),
                                 start=False, stop=False)

            def camm(t, cur):
                nc.tensor.matmul(
                    yt[t][:, 0:CK], w64[:, S * t:S * (t + 1)],
                    cur[0:H, CK - 1:2 * CK - 1],
                    start=False, stop=False, skip_group_check=True)
                nc.tensor.matmul(
                    yt[t][:, CK:2 * CK], w64[:, S * t:S * (t + 1)],
                    xb1[:, :], start=False, stop=True, skip_group_check=True)

            sw_off = {}
            _off = 0
            for _li, (_sig, _ks) in enumerate(SCAN_LEVELS):
                for _k in _ks:
                    sw_off[(_li, _k)] = _off
                    _off += S

            def scan_level(d, cur, nxt):
                sig, ks = SCAN_LEVELS[d]
                ps = wpool.tile([S, CK], F32, tag="w", name=f"scan{d}")
                for j, k in enumerate(ks):
                    sh = k * sig
                    w = sw_off[(d, k)]
                    nc.tensor.matmul(ps[:], w128b[:, w:w + S],
                                     cur[:, CK - sh:2 * CK - sh],
                                     start=(j == 0), stop=(j == len(ks) - 1))
                return ps

            def scan_add(ps, cur, nxt):
                nc.vector.tensor_add(nxt[:, CK:2 * CK], ps[:],
                                     cur[:, CK:2 * CK])

            # fill while the seed copy lands
            qt[0] = wpool.tile([S, COLS], F32, tag="w", name="q0")
            intra(0, 0, True, False)
            intra(0, 1, False, True)
            nc.scalar.copy(zseg(0), qt[0][0:H, :])
            nc.vector.tensor_copy(zseg(1), qt[0][H:S, :])
            qt[1] = wpool.tile([S, COLS], F32, tag="w", name="q1")
            for s in range(4):
                intra(1, s, s == 0, s == 3)
            nc.scalar.copy(zseg(2), qt[1][0:H, :])
            nc.scalar.copy(zseg(3), qt[1][H:S, :])

            # ---- scan levels with interleaved fill work ----
            def mkq2():
                qt[2] = wpool.tile([S, COLS], F32, tag="w", name="q2")
            fill = [mkq2,
                    lambda: intra(2, 0, True, False),
                    lambda: intra(2, 1, False, False),
                    lambda: intra(2, 2, False, False),
                    lambda: intra(2, 3, False, False),
                    lambda: intra(2, 4, False, False),
                    lambda: intra(2, 5, False, True),
                    lambda: nc.scalar.copy(zseg(4), qt[2][0:H, :]),
                    lambda: nc.scalar.copy(zseg(5), qt[2][H:S, :]),
                    lambda: dmm(0, ypool),
                    lambda: dmm(1, ypool),
                    lambda: dmm(2, ypool),
                    lambda: dmm(3, ypool),
                    lambda: dmm(4, ypool),
                    lambda: cmm(0),
                    lambda: cmm(1),
                    lambda: cmm(2),
                    lambda: cmm(3),
                    lambda: cmm(4)]
            nlev = len(SCAN_LEVELS)
            per = [len(fill) // nlev + (1 if i < len(fill) % nlev else 0)
                   for i in range(nlev)]
            cur, nxt = sa, sb
            fi = 0
            for d in range(nlev):
                ps = scan_level(d, cur, nxt)
                mid = fi + max(1, per[d] // 2 + 0)
                while fi < min(mid, len(fill)):
                    fill[fi]()
                    fi += 1
                scan_add(ps, cur, nxt)
                end = (d + 1) * len(fill) // nlev
                while fi < end:
                    fill[fi]()
                    fi += 1
                cur, nxt = nxt, cur
            while fi < len(fill):
                fill[fi]()
                fi += 1
            xfin = cur  # full prefix states x_end[c]
            x3buf = nxt  # level-3 output: final for chunks c < 64

            # ---- finish y: + C dA^{t+1} x_end[c-1] ----
            _yeng = [nc.scalar.copy, nc.vector.tensor_copy]

            def ycopy(t):
                _yeng[t % 2](yseg(t), yt[t][:])

            # batch-1 xprev window down to partitions 0:64; the first 64
            # columns are already final after level 3 (copied early)
            nc.vector.tensor_copy(xb1[:, 0:H], x3buf[H:S, CK - 1:CK - 1 + H])
            nc.vector.tensor_copy(xb1[:, H:CK], xfin[H:S, CK - 1 + H:2 * CK - 1])
            for t in range(5):
                camm(t, xfin)
            for t in range(5):
                ycopy(t)
            for t in (5, 6, 7):
                dmm(t, wpool)
                cmm(t)
                camm(t, xfin)
                ycopy(t)
            # 4 output DMAs of 2 blocks each; SP gets t0-3, Act t4-7
            nc.sync.dma_start(y_d[:, 0:2 * COLS], y_sb[:, 0:2 * COLS])
            nc.sync.dma_start(y_d[:, 2 * COLS:4 * COLS], y_sb[:, 2 * COLS:4 * COLS])
            nc.scalar.dma_start(y_d[:, 4 * COLS:6 * COLS], y_sb[:, 4 * COLS:6 * COLS])
            nc.scalar.dma_start(y_d[:, 6 * COLS:8 * COLS], y_sb[:, 6 * COLS:8 * COLS])

    _split_multi_waits(nc)
    return nc


_NC_CACHE = {}


def _get_nc():
    if "nc" not in _NC_CACHE:
        _NC_CACHE["nc"] = _build()
    return _NC_CACHE["nc"]


def _host_precompute(Bmat, Cmat, Dvec, log_dt):
    Bm = np.asarray(Bmat, dtype=np.float64)
    Cm = np.asarray(Cmat, dtype=np.float64)
    Dv = np.asarray(Dvec, dtype=np.float64)
    x = np.float64(log_dt)
    dt = np.clip(np.logaddexp(0.0, x), DT_MIN, DT_MAX)   # softplus, clipped
    A = -np.eye(H) + np.eye(H, k=-1)
    back = np.eye(H) - 0.5 * dt * A
    fwd = np.eye(H) + 0.5 * dt * A
    dA = np.linalg.solve(back, fwd)
    dB = np.linalg.solve(back, dt * Bm)                  # (H, S)

    G = [dB]
    for _ in range(1, T):
        G.append(dA @ G[-1])
    A8 = np.linalg.matrix_power(dA, T)

    w128 = np.zeros((S, W128_COLS), dtype=np.float64)
    for d in range(T):                                   # GTX blocks 1..8
        w128[:, H * (d + 1): H * (d + 2)] = G[d].T
    w128[:, DD_OFF:DD_OFF + S] = np.diag(Dv)

    w64 = np.zeros((H, W64_COLS), dtype=np.float64)
    dApow = dA.copy()                                    # dA^{t+1}
    for t in range(T):
        if t:
            dApow = dApow @ dA
        w64[:, S * t:S * (t + 1)] = (Cm @ dApow).T       # CAT blocks
    w64[:, CT_OFF:CT_OFF + S] = Cm.T

    w128b = np.zeros((S, W128B_COLS), dtype=np.float64)
    off = 0
    for sig, ks in SCAN_LEVELS:                          # blockdiag SW blocks
        for k in ks:
            wt = np.linalg.matrix_power(A8, k * sig).T
            w128b[0:H, off:off + H] = wt
            w128b[H:S, off + H:off + S] = wt
            off += S
    return {
        "W128": np.ascontiguousarray(w128, dtype=BF_NP),
        "W64": np.ascontiguousarray(w64, dtype=BF_NP),
        "W128B": np.ascontiguousarray(w128b, dtype=BF_NP),
    }


def kernel(u, B, C, D, log_dt, _trace=False):
    pre = _host_precompute(B, C, D, log_dt)
    nc = _get_nc()

    u = np.asarray(u, dtype=np.float32)
    in_maps = []
    for k in range(N_CORES):
        uc = u[k * PB:(k + 1) * PB]                       # (PB, L, S)
        # (b, c, t, i) -> (i, t, b, c) -> (S, NTOK)
        ut = uc.reshape(PB, CK, T, S).transpose(3, 2, 0, 1).reshape(S, NTOK)
        in_maps.append({"u": np.ascontiguousarray(ut).astype(BF_NP), **pre})

    res = run_bass_kernel_spmd(nc, in_maps, core_ids=list(range(N_CORES)),
                               trace=_trace)
    outs = []
    for k in range(N_CORES):
        yk = np.asarray(res.results[k]["y"]).astype(np.float32)  # (S, NTOK)
        # (o, t, b, c) -> (b, c, t, o) -> (PB, L, S)
        yk = yk.reshape(S, T, PB, CK).transpose(2, 3, 1, 0).reshape(PB, L, S)
        outs.append(yk)
    y = np.concatenate(outs, axis=0)
    if _trace:
        kernel.last_result = res
    return y

